# revision 13
# baseline (speedup 1.0000x reference)
"""Causal multi-head self-attention on 8 Trainium2 NeuronCores.

Sharding: batch x head-group. Core c handles batch c//2 and head-group c%2
(8 of the 16 heads), processed as 4 head-pairs through a 2-head-wide
attention pipeline. Each core returns a partial [S, D] output (its 512-dim
slice of the output-projection contraction); the host sums the 2 partials
per batch. vs pure head-TP this cuts per-core DMA 4x: X^T load is one
batch (4.2MB) instead of four, y writeback is [S,D] (8.4MB) instead of
[B,S,D] (33.5MB).

On-device layout strategy (everything stays transposed until the end):
  - qkvT = W_shard @ X^T computed as matmul(lhsT=W^T tile, rhs=X^T tile)
    -> Q^T/K^T/V^T tiles [dv-part, seq-free]; head0 on partitions 0-63,
    head1 on 64-127. X^T resident in SBUF for the whole kernel.
  - scoresT[kpos, q] = matmul(lhsT=K^T tile, rhs=Q^T tile); the two heads
    run concurrently on the PE array via row-tiling (contraction dv=64).
  - causal handling: only q >= kpos tiles/columns are computed (partial-
    width matmuls); the 128-wide diagonal block gets a 0/1 upper-triangle
    mask multiplied in after the exp.
  - softmax without max-subtraction (scores ~ N(0,1): exp is safe in fp32);
    exp on the scalar engine reads PSUM directly, one call for both heads.
  - V is re-transposed to natural layout with PE transposes; an extra
    all-ones column is appended so the attn@V matmul also produces the
    softmax denominators in PSUM row 64 for free.
  - normalization: denominators for all 8 (j, head) tiles of a head-pair
    are collected into one [16, 256] tile and reciprocal'd in ONE DVE call
    (DVE reciprocal is iterative ~8cyc/elem/lane; per-lane free-dim is the
    cost, so spread 4096 denominators over 16 partitions), then DRAM-
    bounce partition-broadcast and one tensor-tensor multiply per tile.
  - output projection y[s,dm] = matmul(lhsT=outT tile [e=128, s],
    rhs=W_out^T shard), accumulating the 4 head-pairs' k=128 contractions
    in PSUM; evacuated by the vector engine (scalar engine stays
    exp-only) and DMA'd out per 128-row block.
"""

import numpy as np

import concourse.bacc as bacc
import concourse.bass as bass
import concourse.mybir as mybir
import concourse.tile as tile

FP32 = mybir.dt.float32

B = 4
S = 2048
D = 1024
H = 16
DV = 64
N_CORES = 8
HEADS_PER_CORE = 8
N_HP = 4                               # head-pairs per core
E = 128                                # rows of Q/K/V per head-pair
EG = HEADS_PER_CORE * DV               # 512 rows of Q/K/V per core

# PE matmul operand dtype. Measured on HW (256x [128,128]x[128,512] MMs):
#   float32:  933 ns/MM; float32r: 352 ns/MM; float16: 284 ns/MM.
MM_DT = mybir.dt.float16

SQ = 512            # q tile width (PSUM bank)
SK = 128            # kpos tile width (contraction)
N_SQ = S // SQ      # 4 q-tiles per head
N_SK = S // SK      # 16 kpos tiles
N_D = D // 128      # 8 contraction tiles for the projections
# [V_h0(64) | 1 | pad(15) | V_h1(64) | 1 | pad(15)] per kpos tile; the pads
# keep each head block's SBUF offset 32B-aligned for the xbar transpose DMA
HBLK = 80
VBLK = 2 * HBLK  # 160


def build_nc() -> bass.Bass:
    # Bacc (not plain Bass): its compile() pass splits multi-wait
    # instructions that walrus codegen otherwise rejects ("Too many sync
    # wait commands" — the ISA has one wait slot per instruction).
    nc = bacc.Bacc(None, target_bir_lowering=False)

    xt = nc.declare_dram_parameter("xt", [D, S], MM_DT, isOutput=False)
    wq = nc.declare_dram_parameter("wq", [128, N_D, EG], MM_DT, isOutput=False)
    wk = nc.declare_dram_parameter("wk", [128, N_D, EG], MM_DT, isOutput=False)
    wv = nc.declare_dram_parameter("wv", [128, N_D, EG], MM_DT, isOutput=False)
    wout = nc.declare_dram_parameter("wout", [128, N_HP, D], MM_DT, isOutput=False)
    y = nc.declare_dram_parameter("y", [S, D], FP32, isOutput=True)

    with tile.TileContext(nc) as tc:
        _build(tc, xt, wq, wk, wv, wout, y)
    nc.compile()
    return nc


def _build(tc, xt, wq, wk, wv, wout, y):
    nc = tc.nc

    def mm(out, lhsT, rhs, start, stop):
        nc.tensor.matmul(out, lhsT=lhsT, rhs=rhs, start=start, stop=stop)

    with (
        tc.tile_pool(name="consts", bufs=1) as consts,
        tc.tile_pool(name="xtp", bufs=1) as xtp,
        tc.tile_pool(name="qkp", bufs=2) as qkp,
        tc.tile_pool(name="vtp", bufs=2) as vtp,
        tc.tile_pool(name="vnp", bufs=2) as vnp,
        tc.tile_pool(name="attnp", bufs=4) as attnp,
        tc.tile_pool(name="outp", bufs=1) as outp,
        tc.tile_pool(name="avstp", bufs=6) as avstp,
        tc.tile_pool(name="denp", bufs=2) as denp,
        tc.tile_pool(name="bcp", bufs=3) as bcp,
        tc.tile_pool(name="ystp", bufs=3) as ystp,
        tc.tile_pool(name="dramp", bufs=2, space="DRAM") as dramp,
        tc.tile_pool(name="ps_work", bufs=2, space="PSUM") as ps_work,
        tc.tile_pool(name="ps_scores", bufs=2, space="PSUM") as ps_scores,
        tc.tile_pool(name="ps_av", bufs=2, space="PSUM") as ps_av,
    ):
        # ---- constants ----
        # mask_tri01[p, c] = 1 if c >= p else 0 (valid where q-col >= kpos-row)
        mask_tri01 = consts.tile([128, 128], MM_DT)
        nc.gpsimd.memset(mask_tri01, 1.0)
        nc.gpsimd.affine_select(
            out=mask_tri01, in_=mask_tri01,
            compare_op=mybir.AluOpType.is_ge,
            fill=0.0, base=0, pattern=[[1, 128]], channel_multiplier=-1,
        )

        # weights (host pre-arranged to lhsT layout, contiguous DMAs).
        # DMA order tuned for startup latency: wq (split per d-tile) and
        # X^T first so the q-projection's first matmuls can start ~2us in;
        # wk/wv/wout arrive under the q-projection's compute.
        w_sb = {}
        for name, w in (("q", wq), ("k", wk), ("v", wv)):
            w_sb[name] = consts.tile(
                [128, N_D, EG], MM_DT, tag=f"w{name}_sb", name=f"w{name}_sb"
            )
        nc.sync.dma_start(out=w_sb["q"], in_=wq[:])
        xt_sb = [
            xtp.tile([128, S], MM_DT, tag=f"xt{t}", name=f"xt_sb{t}")
            for t in range(N_D)
        ]
        for t in range(N_D):
            nc.sync.dma_start(out=xt_sb[t], in_=xt[128 * t:128 * (t + 1), :])
        nc.sync.dma_start(out=w_sb["k"], in_=wk[:])
        nc.sync.dma_start(out=w_sb["v"], in_=wv[:])
        wout_sb = consts.tile([128, N_HP, D], MM_DT)
        nc.sync.dma_start(out=wout_sb, in_=wout[:])
        # fp32 ones source for the ones-columns of v_sb
        ones32 = consts.tile([128, 32], FP32)
        nc.gpsimd.memset(ones32, 1.0)

        # ---- PE clock warmup ----
        # HAM gates the PE at 1.2 GHz until ~3.4us of sustained matmul
        # activity. Run junk matmuls on the mask tile during the initial
        # DMA window so the real matmuls start at 2.4 GHz.
        for r in range(40):
            wps = ps_work.tile([128, 128], FP32, tag="ps_work", name=f"warm{r}")
            mm(wps, lhsT=mask_tri01, rhs=mask_tri01, start=True, stop=True)

        outT = []
        for hp in range(N_HP):
            e0 = E * hp  # this head-pair's rows within the core's EG

            # ---- QKV projections (transposed layout) ----
            scope_qkv = nc.named_scope(f"qkv{hp}"); scope_qkv.__enter__()
            qT_sb = qkp.tile([128, S], MM_DT, tag="qT")
            kT_sb = qkp.tile([128, S], MM_DT, tag="kT")
            vT_sb = vtp.tile([128, S], MM_DT)
            for wname, dst in (("q", qT_sb), ("k", kT_sb), ("v", vT_sb)):
                for j in range(N_SQ):
                    ps = ps_work.tile([128, SQ], FP32, tag="ps_work")
                    for d in range(N_D):
                        mm(
                            ps,
                            lhsT=w_sb[wname][:, d, e0:e0 + E],
                            rhs=xt_sb[d][:, bass.ts(j, SQ)],
                            start=(d == 0),
                            stop=(d == N_D - 1),
                        )
                    nc.vector.tensor_copy(dst[:, bass.ts(j, SQ)], ps)

            # ---- V -> natural layout with ones columns ----
            # v_sb block i: [V_h0(64) | 1 | pad | V_h1(64) | 1 | pad];
            # the transpose runs on the DMA xbar (not the PE)
            v_sb = vnp.tile([128, N_SK, VBLK], MM_DT)
            ones_ap = bass.AP(
                tensor=v_sb.tensor,
                offset=v_sb.offset + DV,
                ap=[v_sb.ap[0], [VBLK, N_SK], [HBLK, 2]],
            )
            nc.vector.tensor_copy(
                ones_ap,
                bass.AP(
                    tensor=ones32.tensor,
                    offset=ones32.offset,
                    ap=[ones32.ap[0], [2, N_SK], [1, 2]],
                ),
            )
            for i in range(N_SK):
                for h in range(2):
                    vdst = bass.AP(
                        tensor=v_sb.tensor,
                        offset=v_sb.offset + i * VBLK + h * HBLK,
                        ap=[v_sb.ap[0], [1, DV]],
                    )
                    nc.sync.dma_start(
                        out=vdst,
                        in_=vT_sb[DV * h:DV * (h + 1), bass.ts(i, SK)],
                        transpose=True,
                    )

            scope_qkv.__exit__(None, None, None)

            # ---- attention ----
            scope_att = nc.named_scope(f"attn{hp}"); scope_att.__enter__()
            outT_full = outp.tile([128, S], MM_DT, tag=f"outT{hp}")
            outT_h1 = outp.tile([64, S], MM_DT, tag="outT_h1", bufs=2)
            for j in range(N_SQ):
                av_ps = [
                    ps_av.tile([DV + 1, SQ], FP32, tag="av_ps", name=f"av_ps{h}")
                    for h in range(2)
                ]
                n_i = 4 * j + 4
                for i in range(n_i):
                    s0 = max(0, SK * i - SQ * j)  # first valid col in q block
                    w = SQ - s0
                    sc_ps = ps_scores.tile([128, 2 * SQ], FP32, tag="sc_ps")
                    for h in range(2):
                        mm(
                            sc_ps[:, SQ * h + s0:SQ * (h + 1)],
                            lhsT=kT_sb[DV * h:DV * (h + 1), bass.ts(i, SK)],
                            rhs=qT_sb[DV * h:DV * (h + 1), SQ * j + s0:SQ * (j + 1)],
                            start=True,
                            stop=True,
                        )
                    attnT = attnp.tile([128, 2 * SQ], MM_DT)
                    # one exp over both heads' partial-width blocks
                    src = bass.AP(
                        tensor=sc_ps.tensor,
                        offset=sc_ps.offset + s0,
                        ap=[sc_ps.ap[0], [SQ, 2], [1, w]],
                    )
                    dst = bass.AP(
                        tensor=attnT.tensor,
                        offset=attnT.offset + s0,
                        ap=[attnT.ap[0], [SQ, 2], [1, w]],
                    )
                    nc.scalar.activation(dst, src, mybir.ActivationFunctionType.Exp)
                    if i >= 4 * j:  # diagonal: zero the upper-triangle entries
                        blk = bass.AP(
                            tensor=attnT.tensor,
                            offset=attnT.offset + s0,
                            ap=[attnT.ap[0], [SQ, 2], [1, 128]],
                        )
                        mask2 = bass.AP(
                            tensor=mask_tri01.tensor,
                            offset=mask_tri01.offset,
                            ap=[mask_tri01.ap[0], [0, 2], [1, 128]],
                        )
                        nc.vector.tensor_mul(blk, blk, mask2)
                    for h in range(2):
                        mm(
                            av_ps[h][:, s0:SQ],
                            lhsT=v_sb[:, i, h * HBLK:h * HBLK + DV + 1],
                            rhs=attnT[:, SQ * h + s0:SQ * (h + 1)],
                            start=(i == 0),
                            stop=(i == n_i - 1),
                        )

                # evacuate the av psums, then normalize this j-tile:
                # denominators (row 64) of both heads -> one [4, 256]
                # reciprocal -> DRAM-bounce partition-broadcast -> multiply
                av_st = []
                for h in range(2):
                    st = avstp.tile(
                        [DV + 1, SQ], FP32, tag="av_st", name=f"av_st{j}_{h}"
                    )
                    nc.vector.tensor_copy(st, av_ps[h])
                    av_st.append(st)
                den = denp.tile([4, 256], FP32, tag="den")
                for h in range(2):
                    for half in range(2):
                        nc.sync.dma_start(
                            out=den[2 * h + half:2 * h + half + 1, :],
                            in_=av_st[h][DV:DV + 1, 256 * half:256 * (half + 1)],
                        )
                recip = denp.tile([4, 256], FP32, tag="recip")
                nc.vector.reciprocal(recip, den)
                rb = dramp.tile([4, 256], FP32, tag="rb")
                nc.sync.dma_start(out=rb, in_=recip)
                for h in range(2):
                    bcast = bcp.tile([DV, SQ], FP32, tag="bcast")
                    nc.gpsimd.dma_start(
                        out=bcast,
                        in_=bass.AP(
                            tensor=rb.tensor,
                            offset=rb.offset + 2 * h * 256,
                            ap=[[0, DV], [256, 2], [1, 256]],
                        ),
                    )
                    dst = (
                        outT_full[0:DV, bass.ts(j, SQ)]
                        if h == 0
                        else outT_h1[:, bass.ts(j, SQ)]
                    )
                    nc.vector.tensor_mul(dst, av_st[h][0:DV, :], bcast)
                # shift head1 rows of this j-tile to partitions 64..127
                nc.gpsimd.dma_start(
                    out=outT_full[DV:128, bass.ts(j, SQ)],
                    in_=outT_h1[:, bass.ts(j, SQ)],
                )
            outT.append(outT_full)

            scope_att.__exit__(None, None, None)

        # ---- output projection (all 4 head-pairs accumulate in PSUM) ----
        scope_y = nc.named_scope("yproj"); scope_y.__enter__()
        for t in range(S // 128):
            yst = ystp.tile([128, D], FP32)
            for n in range(D // SQ):
                yps = ps_scores.tile([128, SQ], FP32, tag="sc_ps")
                for hp in range(N_HP):
                    mm(
                        yps,
                        lhsT=outT[hp][:, bass.ts(t, 128)],
                        rhs=wout_sb[:, hp, bass.ts(n, SQ)],
                        start=(hp == 0),
                        stop=(hp == N_HP - 1),
                    )
                nc.vector.tensor_copy(yst[:, bass.ts(n, SQ)], yps)
            nc.sync.dma_start(out=y[128 * t:128 * (t + 1), :], in_=yst)
        scope_y.__exit__(None, None, None)


def shard_inputs(X, W_qkv, W_out):
    """Host-side sharding. Returns per-core input maps."""
    X = np.asarray(X, dtype=np.float32)
    W_qkv = np.asarray(W_qkv, dtype=np.float32)
    W_out = np.asarray(W_out, dtype=np.float32)
    np_mm = mybir.dt.np(MM_DT)
    xt = np.ascontiguousarray(X.transpose(0, 2, 1)).astype(np_mm)  # [B, D, S]
    scale = np.float32(1.0 / np.sqrt(DV))

    def prep_w(w):  # [EG, D] -> [128, N_D, EG] lhsT tiles
        return np.ascontiguousarray(
            w.T.reshape(N_D, 128, EG).transpose(1, 0, 2)
        ).astype(np_mm)

    shards = []
    for hg in range(2):
        r = slice(EG * hg, EG * (hg + 1))
        shards.append({
            "wq": prep_w(W_qkv[0 * D:1 * D][r] * scale),
            "wk": prep_w(W_qkv[1 * D:2 * D][r]),
            "wv": prep_w(W_qkv[2 * D:3 * D][r]),
            "wout": np.ascontiguousarray(
                W_out[:, r].T.reshape(N_HP, 128, D).transpose(1, 0, 2)
            ).astype(np_mm),
        })
    in_maps = []
    for c in range(N_CORES):
        b, hg = c // 2, c % 2
        m = {"xt": xt[b]}
        m.update(shards[hg])
        in_maps.append(m)
    return in_maps


def gather_output(results):
    """Sum the two head-group partials per batch."""
    out = np.zeros((B, S, D), dtype=np.float32)
    for b in range(B):
        out[b] = results[2 * b]["y"] + results[2 * b + 1]["y"]
    return out


def kernel(X, W_qkv, W_out):
    from concourse.bass_utils import run_bass_kernel_spmd

    nc = build_nc()
    in_maps = shard_inputs(X, W_qkv, W_out)
    res = run_bass_kernel_spmd(nc, in_maps, core_ids=list(range(N_CORES)))
    return gather_output(res.results)


# revision 15
# speedup vs baseline: 1.3915x; 1.3915x over previous
"""Causal multi-head self-attention on 8 Trainium2 NeuronCores.

Sharding: batch x head-group. Core c handles batch c//2 and head-group c%2
(8 of the 16 heads), processed as 4 head-pairs through a 2-head-wide
attention pipeline. Each core returns a partial [S, D] output (its 512-dim
slice of the output-projection contraction); the host sums the 2 partials
per batch. vs pure head-TP this cuts per-core DMA 4x: X^T load is one
batch (4.2MB) instead of four, y writeback is [S,D] (8.4MB) instead of
[B,S,D] (33.5MB).

On-device layout strategy (everything stays transposed until the end):
  - qkvT = W_shard @ X^T computed as matmul(lhsT=W^T tile, rhs=X^T tile)
    -> Q^T/K^T/V^T tiles [dv-part, seq-free]; head0 on partitions 0-63,
    head1 on 64-127. X^T resident in SBUF for the whole kernel.
  - scoresT[kpos, q] = matmul(lhsT=K^T tile, rhs=Q^T tile); the two heads
    run concurrently on the PE array via row-tiling (contraction dv=64).
  - causal handling: only q >= kpos tiles/columns are computed (partial-
    width matmuls); the 128-wide diagonal block gets a 0/1 upper-triangle
    mask multiplied in after the exp.
  - softmax without max-subtraction (scores ~ N(0,1): exp is safe in fp32);
    exp on the scalar engine reads PSUM directly, one call for both heads.
  - V is re-transposed to natural layout with PE transposes; an extra
    all-ones column is appended so the attn@V matmul also produces the
    softmax denominators in PSUM row 64 for free.
  - normalization: denominators for all 8 (j, head) tiles of a head-pair
    are collected into one [16, 256] tile and reciprocal'd in ONE DVE call
    (DVE reciprocal is iterative ~8cyc/elem/lane; per-lane free-dim is the
    cost, so spread 4096 denominators over 16 partitions), then DRAM-
    bounce partition-broadcast and one tensor-tensor multiply per tile.
  - output projection y[s,dm] = matmul(lhsT=outT tile [e=128, s],
    rhs=W_out^T shard), accumulating the 4 head-pairs' k=128 contractions
    in PSUM; evacuated by the vector engine (scalar engine stays
    exp-only) and DMA'd out per 128-row block.
"""

import numpy as np

import concourse.bacc as bacc
import concourse.bass as bass
import concourse.mybir as mybir
import concourse.tile as tile

FP32 = mybir.dt.float32

B = 4
S = 2048
D = 1024
H = 16
DV = 64
N_CORES = 8
HEADS_PER_CORE = 8
N_HP = 4                               # head-pairs per core
E = 128                                # rows of Q/K/V per head-pair
EG = HEADS_PER_CORE * DV               # 512 rows of Q/K/V per core

# PE matmul operand dtype. Measured on HW (256x [128,128]x[128,512] MMs):
#   float32:  933 ns/MM; float32r: 352 ns/MM; float16: 284 ns/MM.
MM_DT = mybir.dt.float16

SQ = 512            # q tile width (PSUM bank)
SK = 128            # kpos tile width (contraction)
N_SQ = S // SQ      # 4 q-tiles per head
N_SK = S // SK      # 16 kpos tiles
N_D = D // 128      # 8 contraction tiles for the projections
VBLK = 2 * (DV + 1)  # 130: [V_h0 | 1 | V_h1 | 1] per kpos tile


def build_nc() -> bass.Bass:
    # Bacc (not plain Bass): its compile() pass splits multi-wait
    # instructions that walrus codegen otherwise rejects ("Too many sync
    # wait commands" — the ISA has one wait slot per instruction).
    nc = bacc.Bacc(None, target_bir_lowering=False)

    xt = nc.declare_dram_parameter("xt", [D, S], MM_DT, isOutput=False)
    wq = nc.declare_dram_parameter("wq", [128, N_D, EG], MM_DT, isOutput=False)
    wk = nc.declare_dram_parameter("wk", [128, N_D, EG], MM_DT, isOutput=False)
    wv = nc.declare_dram_parameter("wv", [128, N_D, EG], MM_DT, isOutput=False)
    wout = nc.declare_dram_parameter("wout", [128, N_HP, D], MM_DT, isOutput=False)
    y = nc.declare_dram_parameter("y", [S, D], FP32, isOutput=True)

    with tile.TileContext(nc) as tc:
        _build(tc, xt, wq, wk, wv, wout, y)
    nc.compile()
    return nc


def _build(tc, xt, wq, wk, wv, wout, y):
    nc = tc.nc

    def mm(out, lhsT, rhs, start, stop):
        nc.tensor.matmul(out, lhsT=lhsT, rhs=rhs, start=start, stop=stop)

    with (
        tc.tile_pool(name="consts", bufs=1) as consts,
        tc.tile_pool(name="xtp", bufs=1) as xtp,
        tc.tile_pool(name="qkp", bufs=2) as qkp,
        tc.tile_pool(name="vtp", bufs=2) as vtp,
        tc.tile_pool(name="vnp", bufs=2) as vnp,
        tc.tile_pool(name="attnp", bufs=4) as attnp,
        tc.tile_pool(name="outp", bufs=1) as outp,
        tc.tile_pool(name="avstp", bufs=6) as avstp,
        tc.tile_pool(name="denp", bufs=2) as denp,
        tc.tile_pool(name="bcp", bufs=3) as bcp,
        tc.tile_pool(name="ystp", bufs=3) as ystp,
        tc.tile_pool(name="dramp", bufs=2, space="DRAM") as dramp,
        tc.tile_pool(name="ps_work", bufs=2, space="PSUM") as ps_work,
        tc.tile_pool(name="ps_scores", bufs=2, space="PSUM") as ps_scores,
        tc.tile_pool(name="ps_av", bufs=2, space="PSUM") as ps_av,
    ):
        # ---- constants ----
        identity = consts.tile([128, 128], FP32)
        nc.gpsimd.memset(identity, 0.0)
        nc.gpsimd.affine_select(
            out=identity, in_=identity,
            compare_op=mybir.AluOpType.not_equal,
            fill=1.0, base=0, pattern=[[-1, 128]], channel_multiplier=1,
        )
        # mask_tri01[p, c] = 1 if c >= p else 0 (valid where q-col >= kpos-row)
        mask_tri01 = consts.tile([128, 128], MM_DT)
        nc.gpsimd.memset(mask_tri01, 1.0)
        nc.gpsimd.affine_select(
            out=mask_tri01, in_=mask_tri01,
            compare_op=mybir.AluOpType.is_ge,
            fill=0.0, base=0, pattern=[[1, 128]], channel_multiplier=-1,
        )

        # weights (host pre-arranged to lhsT layout, contiguous DMAs).
        # DMA order tuned for startup latency: wq (split per d-tile) and
        # X^T first so the q-projection's first matmuls can start ~2us in;
        # wk/wv/wout arrive under the q-projection's compute.
        w_sb = {}
        for name, w in (("q", wq), ("k", wk), ("v", wv)):
            w_sb[name] = consts.tile(
                [128, N_D, EG], MM_DT, tag=f"w{name}_sb", name=f"w{name}_sb"
            )
        nc.sync.dma_start(out=w_sb["q"], in_=wq[:])
        xt_sb = [
            xtp.tile([128, S], MM_DT, tag=f"xt{t}", name=f"xt_sb{t}")
            for t in range(N_D)
        ]
        for t in range(N_D):
            nc.sync.dma_start(out=xt_sb[t], in_=xt[128 * t:128 * (t + 1), :])
        nc.sync.dma_start(out=w_sb["k"], in_=wk[:])
        nc.sync.dma_start(out=w_sb["v"], in_=wv[:])
        wout_sb = consts.tile([128, N_HP, D], MM_DT)
        nc.sync.dma_start(out=wout_sb, in_=wout[:])
        # fp32 ones source for the ones-columns of v_sb
        ones32 = consts.tile([128, 32], FP32)
        nc.gpsimd.memset(ones32, 1.0)

        # ---- PE clock warmup ----
        # HAM gates the PE at 1.2 GHz until ~3.4us of sustained matmul
        # activity. Run junk matmuls on the mask tile during the initial
        # DMA window so the real matmuls start at 2.4 GHz.
        for r in range(40):
            wps = ps_work.tile([128, 128], FP32, tag="ps_work", name=f"warm{r}")
            mm(wps, lhsT=mask_tri01, rhs=mask_tri01, start=True, stop=True)

        outT = []
        for hp in range(N_HP):
            e0 = E * hp  # this head-pair's rows within the core's EG

            # ---- QKV projections (transposed layout) ----
            scope_qkv = nc.named_scope(f"qkv{hp}"); scope_qkv.__enter__()
            qT_sb = qkp.tile([128, S], MM_DT, tag="qT")
            kT_sb = qkp.tile([128, S], MM_DT, tag="kT")
            vT_sb = vtp.tile([128, S], FP32)
            # v_sb block i: [V_h0(64) | 1 | V_h1(64) | 1]
            v_sb = vnp.tile([128, N_SK, VBLK], MM_DT)
            ones_ap = bass.AP(
                tensor=v_sb.tensor,
                offset=v_sb.offset + DV,
                ap=[v_sb.ap[0], [VBLK, N_SK], [DV + 1, 2]],
            )
            nc.vector.tensor_copy(
                ones_ap,
                bass.AP(
                    tensor=ones32.tensor,
                    offset=ones32.offset,
                    ap=[ones32.ap[0], [2, N_SK], [1, 2]],
                ),
            )
            # j-interleaved: each j-round produces the q/k slices and the
            # natural-layout V blocks that attention tile j depends on, so
            # the first scores can start after one round (and the V
            # transposes stay off the head-pair boundary)
            for j in range(N_SQ):
                for wname, dst in (("q", qT_sb), ("k", kT_sb), ("v", vT_sb)):
                    ps = ps_work.tile([128, SQ], FP32, tag="ps_work")
                    for d in range(N_D):
                        mm(
                            ps,
                            lhsT=w_sb[wname][:, d, e0:e0 + E],
                            rhs=xt_sb[d][:, bass.ts(j, SQ)],
                            start=(d == 0),
                            stop=(d == N_D - 1),
                        )
                    nc.vector.tensor_copy(dst[:, bass.ts(j, SQ)], ps)
                for i in range(4 * j, 4 * j + 4):
                    tps = ps_work.tile([128, 128], FP32, tag="ps_work")
                    nc.tensor.transpose(tps, vT_sb[:, bass.ts(i, SK)], identity)
                    vdst = bass.AP(
                        tensor=v_sb.tensor,
                        offset=v_sb.offset + i * VBLK,
                        ap=[v_sb.ap[0], [DV + 1, 2], [1, DV]],
                    )
                    vsrc = bass.AP(
                        tensor=tps.tensor,
                        offset=tps.offset,
                        ap=[tps.ap[0], [DV, 2], [1, DV]],
                    )
                    nc.vector.tensor_copy(vdst, vsrc)

            scope_qkv.__exit__(None, None, None)

            # ---- attention ----
            scope_att = nc.named_scope(f"attn{hp}"); scope_att.__enter__()
            outT_full = outp.tile([128, S], MM_DT, tag=f"outT{hp}")
            outT_h1 = outp.tile([64, S], MM_DT, tag="outT_h1", bufs=2)
            for j in range(N_SQ):
                av_ps = [
                    ps_av.tile([DV + 1, SQ], FP32, tag="av_ps", name=f"av_ps{h}")
                    for h in range(2)
                ]
                n_i = 4 * j + 4
                for i in range(n_i):
                    s0 = max(0, SK * i - SQ * j)  # first valid col in q block
                    w = SQ - s0
                    sc_ps = ps_scores.tile([128, 2 * SQ], FP32, tag="sc_ps")
                    for h in range(2):
                        mm(
                            sc_ps[:, SQ * h + s0:SQ * (h + 1)],
                            lhsT=kT_sb[DV * h:DV * (h + 1), bass.ts(i, SK)],
                            rhs=qT_sb[DV * h:DV * (h + 1), SQ * j + s0:SQ * (j + 1)],
                            start=True,
                            stop=True,
                        )
                    attnT = attnp.tile([128, 2 * SQ], MM_DT)
                    # one exp over both heads' partial-width blocks
                    src = bass.AP(
                        tensor=sc_ps.tensor,
                        offset=sc_ps.offset + s0,
                        ap=[sc_ps.ap[0], [SQ, 2], [1, w]],
                    )
                    dst = bass.AP(
                        tensor=attnT.tensor,
                        offset=attnT.offset + s0,
                        ap=[attnT.ap[0], [SQ, 2], [1, w]],
                    )
                    nc.scalar.activation(dst, src, mybir.ActivationFunctionType.Exp)
                    if i >= 4 * j:  # diagonal: zero the upper-triangle entries
                        blk = bass.AP(
                            tensor=attnT.tensor,
                            offset=attnT.offset + s0,
                            ap=[attnT.ap[0], [SQ, 2], [1, 128]],
                        )
                        mask2 = bass.AP(
                            tensor=mask_tri01.tensor,
                            offset=mask_tri01.offset,
                            ap=[mask_tri01.ap[0], [0, 2], [1, 128]],
                        )
                        nc.vector.tensor_mul(blk, blk, mask2)
                    for h in range(2):
                        mm(
                            av_ps[h][:, s0:SQ],
                            lhsT=v_sb[:, i, h * (DV + 1):(h + 1) * (DV + 1)],
                            rhs=attnT[:, SQ * h + s0:SQ * (h + 1)],
                            start=(i == 0),
                            stop=(i == n_i - 1),
                        )

                # evacuate the av psums, then normalize this j-tile:
                # denominators (row 64) of both heads -> one [4, 256]
                # reciprocal -> DRAM-bounce partition-broadcast -> multiply
                av_st = []
                for h in range(2):
                    st = avstp.tile(
                        [DV + 1, SQ], FP32, tag="av_st", name=f"av_st{j}_{h}"
                    )
                    nc.vector.tensor_copy(st, av_ps[h])
                    av_st.append(st)
                den = denp.tile([4, 256], FP32, tag="den")
                for h in range(2):
                    for half in range(2):
                        nc.sync.dma_start(
                            out=den[2 * h + half:2 * h + half + 1, :],
                            in_=av_st[h][DV:DV + 1, 256 * half:256 * (half + 1)],
                        )
                recip = denp.tile([4, 256], FP32, tag="recip")
                nc.vector.reciprocal(recip, den)
                rb = dramp.tile([4, 256], FP32, tag="rb")
                nc.sync.dma_start(out=rb, in_=recip)
                for h in range(2):
                    bcast = bcp.tile([DV, SQ], FP32, tag="bcast")
                    nc.gpsimd.dma_start(
                        out=bcast,
                        in_=bass.AP(
                            tensor=rb.tensor,
                            offset=rb.offset + 2 * h * 256,
                            ap=[[0, DV], [256, 2], [1, 256]],
                        ),
                    )
                    dst = (
                        outT_full[0:DV, bass.ts(j, SQ)]
                        if h == 0
                        else outT_h1[:, bass.ts(j, SQ)]
                    )
                    nc.vector.tensor_mul(dst, av_st[h][0:DV, :], bcast)
                # shift head1 rows of this j-tile to partitions 64..127
                nc.gpsimd.dma_start(
                    out=outT_full[DV:128, bass.ts(j, SQ)],
                    in_=outT_h1[:, bass.ts(j, SQ)],
                )
            outT.append(outT_full)

            scope_att.__exit__(None, None, None)

        # ---- output projection (all 4 head-pairs accumulate in PSUM) ----
        scope_y = nc.named_scope("yproj"); scope_y.__enter__()
        for t in range(S // 128):
            yst = ystp.tile([128, D], FP32)
            for n in range(D // SQ):
                yps = ps_scores.tile([128, SQ], FP32, tag="sc_ps")
                for hp in range(N_HP):
                    mm(
                        yps,
                        lhsT=outT[hp][:, bass.ts(t, 128)],
                        rhs=wout_sb[:, hp, bass.ts(n, SQ)],
                        start=(hp == 0),
                        stop=(hp == N_HP - 1),
                    )
                nc.vector.tensor_copy(yst[:, bass.ts(n, SQ)], yps)
            nc.sync.dma_start(out=y[128 * t:128 * (t + 1), :], in_=yst)
        scope_y.__exit__(None, None, None)


def shard_inputs(X, W_qkv, W_out):
    """Host-side sharding. Returns per-core input maps."""
    X = np.asarray(X, dtype=np.float32)
    W_qkv = np.asarray(W_qkv, dtype=np.float32)
    W_out = np.asarray(W_out, dtype=np.float32)
    np_mm = mybir.dt.np(MM_DT)
    xt = np.ascontiguousarray(X.transpose(0, 2, 1)).astype(np_mm)  # [B, D, S]
    scale = np.float32(1.0 / np.sqrt(DV))

    def prep_w(w):  # [EG, D] -> [128, N_D, EG] lhsT tiles
        return np.ascontiguousarray(
            w.T.reshape(N_D, 128, EG).transpose(1, 0, 2)
        ).astype(np_mm)

    shards = []
    for hg in range(2):
        r = slice(EG * hg, EG * (hg + 1))
        shards.append({
            "wq": prep_w(W_qkv[0 * D:1 * D][r] * scale),
            "wk": prep_w(W_qkv[1 * D:2 * D][r]),
            "wv": prep_w(W_qkv[2 * D:3 * D][r]),
            "wout": np.ascontiguousarray(
                W_out[:, r].T.reshape(N_HP, 128, D).transpose(1, 0, 2)
            ).astype(np_mm),
        })
    in_maps = []
    for c in range(N_CORES):
        b, hg = c // 2, c % 2
        m = {"xt": xt[b]}
        m.update(shards[hg])
        in_maps.append(m)
    return in_maps


def gather_output(results):
    """Sum the two head-group partials per batch."""
    out = np.zeros((B, S, D), dtype=np.float32)
    for b in range(B):
        out[b] = results[2 * b]["y"] + results[2 * b + 1]["y"]
    return out


def kernel(X, W_qkv, W_out):
    from concourse.bass_utils import run_bass_kernel_spmd

    nc = build_nc()
    in_maps = shard_inputs(X, W_qkv, W_out)
    res = run_bass_kernel_spmd(nc, in_maps, core_ids=list(range(N_CORES)))
    return gather_output(res.results)


# revision 20
# speedup vs baseline: 1.4045x; 1.0093x over previous
"""Causal multi-head self-attention on 8 Trainium2 NeuronCores.

Sharding: batch x head-group. Core c handles batch c//2 and head-group c%2
(8 of the 16 heads), processed as 4 head-pairs through a 2-head-wide
attention pipeline. Each core returns a partial [S, D] output (its 512-dim
slice of the output-projection contraction); the host sums the 2 partials
per batch. vs pure head-TP this cuts per-core DMA 4x: X^T load is one
batch (4.2MB) instead of four, y writeback is [S,D] (8.4MB) instead of
[B,S,D] (33.5MB).

On-device layout strategy (everything stays transposed until the end):
  - qkvT = W_shard @ X^T computed as matmul(lhsT=W^T tile, rhs=X^T tile)
    -> Q^T/K^T/V^T tiles [dv-part, seq-free]; head0 on partitions 0-63,
    head1 on 64-127. X^T resident in SBUF for the whole kernel.
  - scoresT[kpos, q] = matmul(lhsT=K^T tile, rhs=Q^T tile); the two heads
    run concurrently on the PE array via row-tiling (contraction dv=64).
  - causal handling: only q >= kpos tiles/columns are computed (partial-
    width matmuls); the 128-wide diagonal block gets a 0/1 upper-triangle
    mask multiplied in after the exp.
  - softmax without max-subtraction (scores ~ N(0,1): exp is safe in fp32);
    exp on the scalar engine reads PSUM directly, one call for both heads.
  - V is re-transposed to natural layout with PE transposes; an extra
    all-ones column is appended so the attn@V matmul also produces the
    softmax denominators in PSUM row 64 for free.
  - normalization: denominators for all 8 (j, head) tiles of a head-pair
    are collected into one [16, 256] tile and reciprocal'd in ONE DVE call
    (DVE reciprocal is iterative ~8cyc/elem/lane; per-lane free-dim is the
    cost, so spread 4096 denominators over 16 partitions), then DRAM-
    bounce partition-broadcast and one tensor-tensor multiply per tile.
  - output projection y[s,dm] = matmul(lhsT=outT tile [e=128, s],
    rhs=W_out^T shard), accumulating the 4 head-pairs' k=128 contractions
    in PSUM; evacuated by the vector engine (scalar engine stays
    exp-only) and DMA'd out per 128-row block.
"""

import numpy as np

import concourse.bacc as bacc
import concourse.bass as bass
import concourse.mybir as mybir
import concourse.tile as tile

FP32 = mybir.dt.float32

B = 4
S = 2048
D = 1024
H = 16
DV = 64
N_CORES = 8
HEADS_PER_CORE = 8
N_HP = 4                               # head-pairs per core
E = 128                                # rows of Q/K/V per head-pair
EG = HEADS_PER_CORE * DV               # 512 rows of Q/K/V per core

# PE matmul operand dtype. Measured on HW (256x [128,128]x[128,512] MMs):
#   float32:  933 ns/MM; float32r: 352 ns/MM; float16: 284 ns/MM.
MM_DT = mybir.dt.float16

SQ = 512            # q tile width (PSUM bank)
SK = 128            # kpos tile width (contraction)
N_SQ = S // SQ      # 4 q-tiles per head
N_SK = S // SK      # 16 kpos tiles
N_D = D // 128      # 8 contraction tiles for the projections
VBLK = 2 * (DV + 1)  # 130: [V_h0 | 1 | V_h1 | 1] per kpos tile


def build_nc() -> bass.Bass:
    # Bacc (not plain Bass): its compile() pass splits multi-wait
    # instructions that walrus codegen otherwise rejects ("Too many sync
    # wait commands" — the ISA has one wait slot per instruction).
    nc = bacc.Bacc(None, target_bir_lowering=False)

    xt = nc.declare_dram_parameter("xt", [D, S], MM_DT, isOutput=False)
    wq = nc.declare_dram_parameter("wq", [128, N_D, EG], MM_DT, isOutput=False)
    wk = nc.declare_dram_parameter("wk", [128, N_D, EG], MM_DT, isOutput=False)
    wv = nc.declare_dram_parameter("wv", [128, N_D, EG], MM_DT, isOutput=False)
    wout = nc.declare_dram_parameter("wout", [128, N_HP, D], MM_DT, isOutput=False)
    y = nc.declare_dram_parameter("y", [S, D], FP32, isOutput=True)

    with tile.TileContext(nc) as tc:
        _build(tc, xt, wq, wk, wv, wout, y)
    nc.compile()
    return nc


def _build(tc, xt, wq, wk, wv, wout, y):
    nc = tc.nc

    def mm(out, lhsT, rhs, start, stop):
        nc.tensor.matmul(out, lhsT=lhsT, rhs=rhs, start=start, stop=stop)

    with (
        tc.tile_pool(name="consts", bufs=1) as consts,
        tc.tile_pool(name="xtp", bufs=1) as xtp,
        tc.tile_pool(name="qkp", bufs=2) as qkp,
        tc.tile_pool(name="vtp", bufs=2) as vtp,
        tc.tile_pool(name="vnp", bufs=2) as vnp,
        tc.tile_pool(name="attnp", bufs=6) as attnp,
        tc.tile_pool(name="outp", bufs=1) as outp,
        tc.tile_pool(name="avstp", bufs=6) as avstp,
        tc.tile_pool(name="denp", bufs=2) as denp,
        tc.tile_pool(name="bcp", bufs=3) as bcp,
        tc.tile_pool(name="ystp", bufs=3) as ystp,
        tc.tile_pool(name="dramp", bufs=2, space="DRAM") as dramp,
        tc.tile_pool(name="ps_work", bufs=2, space="PSUM") as ps_work,
        tc.tile_pool(name="ps_scores", bufs=2, space="PSUM") as ps_scores,
        tc.tile_pool(name="ps_av", bufs=2, space="PSUM") as ps_av,
    ):
        # ---- constants ----
        identity = consts.tile([128, 128], FP32)
        nc.gpsimd.memset(identity, 0.0)
        nc.gpsimd.affine_select(
            out=identity, in_=identity,
            compare_op=mybir.AluOpType.not_equal,
            fill=1.0, base=0, pattern=[[-1, 128]], channel_multiplier=1,
        )
        # mask_tri01[p, c] = 1 if c >= p else 0 (valid where q-col >= kpos-row)
        mask_tri01 = consts.tile([128, 128], MM_DT)
        nc.gpsimd.memset(mask_tri01, 1.0)
        nc.gpsimd.affine_select(
            out=mask_tri01, in_=mask_tri01,
            compare_op=mybir.AluOpType.is_ge,
            fill=0.0, base=0, pattern=[[1, 128]], channel_multiplier=-1,
        )

        # weights (host pre-arranged to lhsT layout, contiguous DMAs).
        # DMA order tuned for startup latency: wq (split per d-tile) and
        # X^T first so the q-projection's first matmuls can start ~2us in;
        # wk/wv/wout arrive under the q-projection's compute.
        w_sb = {}
        for name, w in (("q", wq), ("k", wk), ("v", wv)):
            w_sb[name] = consts.tile(
                [128, N_D, EG], MM_DT, tag=f"w{name}_sb", name=f"w{name}_sb"
            )
        nc.sync.dma_start(out=w_sb["q"], in_=wq[:])
        xt_sb = [
            xtp.tile([128, S], MM_DT, tag=f"xt{t}", name=f"xt_sb{t}")
            for t in range(N_D)
        ]
        for t in range(N_D):
            nc.sync.dma_start(out=xt_sb[t], in_=xt[128 * t:128 * (t + 1), :])
        nc.sync.dma_start(out=w_sb["k"], in_=wk[:])
        nc.sync.dma_start(out=w_sb["v"], in_=wv[:])
        wout_sb = consts.tile([128, N_HP, D], MM_DT)
        nc.sync.dma_start(out=wout_sb, in_=wout[:])
        # fp32 ones source for the ones-columns of v_sb
        ones32 = consts.tile([128, 32], FP32)
        nc.gpsimd.memset(ones32, 1.0)

        # ---- PE clock warmup ----
        # HAM gates the PE at 1.2 GHz until ~3.4us of sustained matmul
        # activity. Run junk matmuls on the mask tile during the initial
        # DMA window so the real matmuls start at 2.4 GHz.
        for r in range(40):
            wps = ps_work.tile([128, 128], FP32, tag="ps_work", name=f"warm{r}")
            mm(wps, lhsT=mask_tri01, rhs=mask_tri01, start=True, stop=True)

        outT = []
        for hp in range(N_HP):
            e0 = E * hp  # this head-pair's rows within the core's EG

            # ---- QKV projections (transposed layout) ----
            scope_qkv = nc.named_scope(f"qkv{hp}"); scope_qkv.__enter__()
            qT_sb = qkp.tile([128, S], MM_DT, tag="qT")
            kT_sb = qkp.tile([128, S], MM_DT, tag="kT")
            vT_sb = vtp.tile([128, S], FP32)
            for wname, dst in (("q", qT_sb), ("k", kT_sb), ("v", vT_sb)):
                for j in range(N_SQ):
                    ps = ps_work.tile([128, SQ], FP32, tag="ps_work")
                    for d in range(N_D):
                        mm(
                            ps,
                            lhsT=w_sb[wname][:, d, e0:e0 + E],
                            rhs=xt_sb[d][:, bass.ts(j, SQ)],
                            start=(d == 0),
                            stop=(d == N_D - 1),
                        )
                    nc.vector.tensor_copy(dst[:, bass.ts(j, SQ)], ps)

            # ---- V -> natural layout with ones columns ----
            # v_sb block i: [V_h0(64) | 1 | V_h1(64) | 1]
            v_sb = vnp.tile([128, N_SK, VBLK], MM_DT)
            ones_ap = bass.AP(
                tensor=v_sb.tensor,
                offset=v_sb.offset + DV,
                ap=[v_sb.ap[0], [VBLK, N_SK], [DV + 1, 2]],
            )
            nc.vector.tensor_copy(
                ones_ap,
                bass.AP(
                    tensor=ones32.tensor,
                    offset=ones32.offset,
                    ap=[ones32.ap[0], [2, N_SK], [1, 2]],
                ),
            )
            for i in range(N_SK):
                tps = ps_work.tile([128, 128], FP32, tag="ps_work")
                nc.tensor.transpose(tps, vT_sb[:, bass.ts(i, SK)], identity)
                vdst = bass.AP(
                    tensor=v_sb.tensor,
                    offset=v_sb.offset + i * VBLK,
                    ap=[v_sb.ap[0], [DV + 1, 2], [1, DV]],
                )
                vsrc = bass.AP(
                    tensor=tps.tensor,
                    offset=tps.offset,
                    ap=[tps.ap[0], [DV, 2], [1, DV]],
                )
                nc.vector.tensor_copy(vdst, vsrc)

            scope_qkv.__exit__(None, None, None)

            # ---- attention ----
            scope_att = nc.named_scope(f"attn{hp}"); scope_att.__enter__()
            outT_full = outp.tile([128, S], MM_DT, tag=f"outT{hp}")
            outT_h1 = outp.tile([64, S], MM_DT, tag="outT_h1", bufs=2)
            for j in range(N_SQ):
                av_ps = [
                    ps_av.tile([DV + 1, SQ], FP32, tag="av_ps", name=f"av_ps{h}")
                    for h in range(2)
                ]
                n_i = 4 * j + 4
                # diagonal tiles first: their exp chains are small and
                # latency-bound, so they overlap the previous j's dense
                # tail; each j then ends with dense full-width matmuls
                order = list(range(4 * j, n_i)) + list(range(0, 4 * j))
                for idx, i in enumerate(order):
                    s0 = max(0, SK * i - SQ * j)  # first valid col in q block
                    w = SQ - s0
                    sc_ps = ps_scores.tile([128, 2 * SQ], FP32, tag="sc_ps")
                    for h in range(2):
                        mm(
                            sc_ps[:, SQ * h + s0:SQ * (h + 1)],
                            lhsT=kT_sb[DV * h:DV * (h + 1), bass.ts(i, SK)],
                            rhs=qT_sb[DV * h:DV * (h + 1), SQ * j + s0:SQ * (j + 1)],
                            start=True,
                            stop=True,
                        )
                    attnT = attnp.tile([128, 2 * SQ], MM_DT)
                    # one exp over both heads' partial-width blocks
                    src = bass.AP(
                        tensor=sc_ps.tensor,
                        offset=sc_ps.offset + s0,
                        ap=[sc_ps.ap[0], [SQ, 2], [1, w]],
                    )
                    dst = bass.AP(
                        tensor=attnT.tensor,
                        offset=attnT.offset + s0,
                        ap=[attnT.ap[0], [SQ, 2], [1, w]],
                    )
                    nc.scalar.activation(dst, src, mybir.ActivationFunctionType.Exp)
                    if i >= 4 * j:  # diagonal: zero the upper-triangle entries
                        blk = bass.AP(
                            tensor=attnT.tensor,
                            offset=attnT.offset + s0,
                            ap=[attnT.ap[0], [SQ, 2], [1, 128]],
                        )
                        mask2 = bass.AP(
                            tensor=mask_tri01.tensor,
                            offset=mask_tri01.offset,
                            ap=[mask_tri01.ap[0], [0, 2], [1, 128]],
                        )
                        nc.vector.tensor_mul(blk, blk, mask2)
                    for h in range(2):
                        mm(
                            av_ps[h][:, s0:SQ],
                            lhsT=v_sb[:, i, h * (DV + 1):(h + 1) * (DV + 1)],
                            rhs=attnT[:, SQ * h + s0:SQ * (h + 1)],
                            start=(idx == 0),
                            stop=(idx == n_i - 1),
                        )

                # evacuate the av psums, then normalize this j-tile:
                # denominators (row 64) of both heads -> one [4, 256]
                # reciprocal -> DRAM-bounce partition-broadcast -> multiply
                av_st = []
                for h in range(2):
                    st = avstp.tile(
                        [DV + 1, SQ], FP32, tag="av_st", name=f"av_st{j}_{h}"
                    )
                    nc.vector.tensor_copy(st, av_ps[h])
                    av_st.append(st)
                den = denp.tile([4, 256], FP32, tag="den")
                for h in range(2):
                    for half in range(2):
                        nc.sync.dma_start(
                            out=den[2 * h + half:2 * h + half + 1, :],
                            in_=av_st[h][DV:DV + 1, 256 * half:256 * (half + 1)],
                        )
                recip = denp.tile([4, 256], FP32, tag="recip")
                nc.vector.reciprocal(recip, den)
                rb = dramp.tile([4, 256], FP32, tag="rb")
                nc.sync.dma_start(out=rb, in_=recip)
                for h in range(2):
                    bcast = bcp.tile([DV, SQ], FP32, tag="bcast")
                    nc.gpsimd.dma_start(
                        out=bcast,
                        in_=bass.AP(
                            tensor=rb.tensor,
                            offset=rb.offset + 2 * h * 256,
                            ap=[[0, DV], [256, 2], [1, 256]],
                        ),
                    )
                    dst = (
                        outT_full[0:DV, bass.ts(j, SQ)]
                        if h == 0
                        else outT_h1[:, bass.ts(j, SQ)]
                    )
                    nc.vector.tensor_mul(dst, av_st[h][0:DV, :], bcast)
                # shift head1 rows of this j-tile to partitions 64..127
                nc.gpsimd.dma_start(
                    out=outT_full[DV:128, bass.ts(j, SQ)],
                    in_=outT_h1[:, bass.ts(j, SQ)],
                )
            outT.append(outT_full)

            scope_att.__exit__(None, None, None)

        # ---- output projection (all 4 head-pairs accumulate in PSUM) ----
        scope_y = nc.named_scope("yproj"); scope_y.__enter__()
        for t in range(S // 128):
            yst = ystp.tile([128, D], FP32)
            for n in range(D // SQ):
                yps = ps_scores.tile([128, SQ], FP32, tag="sc_ps")
                for hp in range(N_HP):
                    mm(
                        yps,
                        lhsT=outT[hp][:, bass.ts(t, 128)],
                        rhs=wout_sb[:, hp, bass.ts(n, SQ)],
                        start=(hp == 0),
                        stop=(hp == N_HP - 1),
                    )
                # alternate PSUM evacuation between DVE and ACT (idle by
                # now) so the projection matmuls never wait on one engine
                if n == 0:
                    nc.vector.tensor_copy(yst[:, bass.ts(n, SQ)], yps)
                else:
                    nc.scalar.copy(yst[:, bass.ts(n, SQ)], yps)
            nc.sync.dma_start(out=y[128 * t:128 * (t + 1), :], in_=yst)
        scope_y.__exit__(None, None, None)


def shard_inputs(X, W_qkv, W_out):
    """Host-side sharding. Returns per-core input maps."""
    X = np.asarray(X, dtype=np.float32)
    W_qkv = np.asarray(W_qkv, dtype=np.float32)
    W_out = np.asarray(W_out, dtype=np.float32)
    np_mm = mybir.dt.np(MM_DT)
    xt = np.ascontiguousarray(X.transpose(0, 2, 1)).astype(np_mm)  # [B, D, S]
    scale = np.float32(1.0 / np.sqrt(DV))

    def prep_w(w):  # [EG, D] -> [128, N_D, EG] lhsT tiles
        return np.ascontiguousarray(
            w.T.reshape(N_D, 128, EG).transpose(1, 0, 2)
        ).astype(np_mm)

    shards = []
    for hg in range(2):
        r = slice(EG * hg, EG * (hg + 1))
        shards.append({
            "wq": prep_w(W_qkv[0 * D:1 * D][r] * scale),
            "wk": prep_w(W_qkv[1 * D:2 * D][r]),
            "wv": prep_w(W_qkv[2 * D:3 * D][r]),
            "wout": np.ascontiguousarray(
                W_out[:, r].T.reshape(N_HP, 128, D).transpose(1, 0, 2)
            ).astype(np_mm),
        })
    in_maps = []
    for c in range(N_CORES):
        b, hg = c // 2, c % 2
        m = {"xt": xt[b]}
        m.update(shards[hg])
        in_maps.append(m)
    return in_maps


def gather_output(results):
    """Sum the two head-group partials per batch."""
    out = np.zeros((B, S, D), dtype=np.float32)
    for b in range(B):
        out[b] = results[2 * b]["y"] + results[2 * b + 1]["y"]
    return out


def kernel(X, W_qkv, W_out):
    from concourse.bass_utils import run_bass_kernel_spmd

    nc = build_nc()
    in_maps = shard_inputs(X, W_qkv, W_out)
    res = run_bass_kernel_spmd(nc, in_maps, core_ids=list(range(N_CORES)))
    return gather_output(res.results)
